# revision 21
# baseline (speedup 1.0000x reference)
"""ByteContextAwareRouter Trainium2 kernel.

Strategy: data-parallel over the N=B*S=8192 token axis across 8 NeuronCores
(1024 tokens/core). Each core runs the full gate pipeline feature-major
(activations stored [feature, token]) so no activation transposes are needed:

  ctx_hidden = gelu(x @ w_c1 + b_c1)        [2048, T]
  ctx        = ctx_hidden @ w_c2 + b_c2     [64, T]
  gi         = rmsnorm([x; ctx; pos_emb])   [1152(pad), T]
  h          = (silu(gi@w1) * (gi@w3)) @ w2 + gi
  logits     = h @ (proj_w / max(temp,.3))  [8, T]

Matmuls run in float32r (full-rate reduced-precision fp32, ~1.4e-4 max rel
err measured on HW) with fp32 PSUM accumulation. The per-token rmsnorm scale
r commutes through the w1/w3 matmuls, so those take the raw resident
[x; ctx; pos] chunks as RHS and r is applied at PSUM eviction — no
materialized gi buffer. Tokens are processed in two 512-token passes so all
intermediates fit in SBUF; weights stream from HBM once per pass. Device
outputs per core: logits [8,1024] and context-feature column sums [96,2].
The tiny routing tail (softmax, top-2, global sorted greedy capacity
dispatch over 16384 pairs, aux loss) replicates the reference exactly on
host in numpy.
"""
import numpy as np

B, S, H = 4, 2048, 1024
CTX, E, K = 64, 8, 2
POS_D = 32
DG = 1120          # H + CTX + POS_D
DGP = 1152         # padded to 9*128
HID = 3072
N = B * S
CAPACITY = 2048
AUX_COEF = 0.01
NCORES = 8
T = N // NCORES    # 1024 tokens per core
TH = 512           # tokens per pass (matmul free dim)
P = 128

_CACHE = {}
USE_FP16 = False


def _build_module(use_fp16=False):
    import concourse.bacc as bacc
    import concourse.mybir as mybir
    from concourse.tile import TileContext

    F32 = mybir.dt.float32
    F16 = mybir.dt.float16
    MMDT = F16 if use_fp16 else mybir.dt.float32r
    AF = mybir.ActivationFunctionType
    ALU = mybir.AluOpType

    KX = H // P          # 8   x feature chunks
    MC1 = 2 * H // P     # 16  ctx-hidden chunks
    KG = DGP // P        # 9   gate-input chunks
    MH = HID // P        # 24  hidden chunks
    MO = DGP // P        # 9   mlp output chunks
    W2T = 8              # w2 k-chunks per streamed tile

    nc = bacc.Bacc("TRN2", target_bir_lowering=False, debug=False,
                   num_devices=NCORES)

    # ---- DRAM parameters (per-core shards / replicated weight blobs) ----
    xT_ext = nc.declare_dram_parameter("xT", [H, T], MMDT, isOutput=False)
    # pos_emb.T padded with zero rows to fill ctxpos partitions 64..127
    posT_ext = nc.declare_dram_parameter("posT", [P - CTX, T], MMDT,
                                         isOutput=False)
    wc1_ext = nc.declare_dram_parameter("wc1b", [MC1, KX, P, P], MMDT,
                                        isOutput=False)
    wc2_ext = nc.declare_dram_parameter("wc2b", [MC1, P, CTX], MMDT,
                                        isOutput=False)
    w1_ext = nc.declare_dram_parameter("w1b", [MH, KG, P, P], MMDT,
                                       isOutput=False)
    w3_ext = nc.declare_dram_parameter("w3b", [MH, KG, P, P], MMDT,
                                       isOutput=False)
    w2_ext = nc.declare_dram_parameter("w2b", [MO, MH, P, P], F16,
                                       isOutput=False)
    proj_ext = nc.declare_dram_parameter("projb", [KG, P, E], MMDT,
                                         isOutput=False)
    ones_ext = nc.declare_dram_parameter("ones", [P, 1], MMDT, isOutput=False)
    bc1_ext = nc.declare_dram_parameter("bc1", [P, MC1], F32, isOutput=False)
    bc2_ext = nc.declare_dram_parameter("bc2", [CTX, 1], F32, isOutput=False)
    norm_ext = nc.declare_dram_parameter("normc", [P, KG], F32,
                                         isOutput=False)
    logits_ext = nc.declare_dram_parameter("logits", [E, T], F32,
                                           isOutput=True)
    ctxsum_ext = nc.declare_dram_parameter("ctxsum", [CTX + POS_D, 2], F32,
                                           isOutput=True)

    from contextlib import ExitStack
    with TileContext(nc) as tc:
        with ExitStack() as stack:
            ep = stack.enter_context
            constp = ep(tc.tile_pool(name="const", bufs=1))
            xp = ep(tc.tile_pool(name="xT", bufs=1))
            gp = ep(tc.tile_pool(name="g", bufs=1))
            hcq = ep(tc.tile_pool(name="hc", bufs=2))
            wc1p = ep(tc.tile_pool(name="wc1s", bufs=2))
            w13p = ep(tc.tile_pool(name="w13s", bufs=3))
            w2p = ep(tc.tile_pool(name="w2s", bufs=3))
            ctxq = ep(tc.tile_pool(name="ctxq", bufs=2))
            chq = ep(tc.tile_pool(name="chq", bufs=2))
            tmpq = ep(tc.tile_pool(name="tmpq", bufs=2))
            suq = ep(tc.tile_pool(name="suq", bufs=2))
            bcq = ep(tc.tile_pool(name="bcq", bufs=2))
            rowsp = ep(tc.tile_pool(name="rows", bufs=1))
            outp = ep(tc.tile_pool(name="out", bufs=1))
            psp = ep(tc.tile_pool(name="psum", bufs=5, space="PSUM"))
            pssp = ep(tc.tile_pool(name="psum_s", bufs=2, space="PSUM"))
            # ---- packed f32 constants: 1:17=bc1, 17=bc2, 18:27=norm ----
            cpack_f = constp.tile([P, 28], F32)
            nc.gpsimd.dma_start(cpack_f[:, 1:1 + MC1], bc1_ext[:])
            nc.gpsimd.dma_start(cpack_f[:CTX, 17:18], bc2_ext[:])
            nc.gpsimd.dma_start(cpack_f[:, 18:18 + KG], norm_ext[:])

            # first two w_c1 tiles ahead of the bulk xT load so the PE can
            # start as soon as the first xT chunk lands
            wc1_pre = []
            for m in range(2):
                wt = wc1p.tile([P, KX, P], MMDT, tag="wc1",
                               name=f"wc1pre{m}")
                nc.sync.dma_start(wt[:],
                                  wc1_ext[m].rearrange("o p m -> p o m"))
                wc1_pre.append(wt)

            xT_ch = xT_ext.rearrange("(o p) t -> o p t", p=P)
            xT_t = xp.tile([P, KX, T], MMDT)
            for k in range(KX):
                nc.gpsimd.dma_start(xT_t[:, k], xT_ch[k])
            xT_f32 = xT_t if use_fp16 else xT_t.bitcast(F32)

            wc2_t = constp.tile([P, MC1, CTX], MMDT)
            nc.gpsimd.dma_start(wc2_t[:], wc2_ext.rearrange("o p n -> p o n"))
            proj_t = constp.tile([P, KG, E], MMDT)
            nc.gpsimd.dma_start(proj_t[:],
                                proj_ext.rearrange("o p n -> p o n"))
            ones_t = constp.tile([P, 1], MMDT)
            nc.gpsimd.dma_start(ones_t[:], ones_ext[:])
            ones_col = ones_t[:]

            logits_sb = outp.tile([E, T], F32)
            ctxsum_sb = outp.tile([CTX + POS_D, 2], F32)

            HSL = [slice(0, TH), slice(TH, 2 * TH)]

            # ---------- phase A: ctx MLP, both halves per weight tile -----
            psb = [pssp.tile([CTX, TH], F32, tag="smps", name=f"psb{i}")
                   for i in range(2)]
            for m in range(MC1):
                if m < 2:
                    wt = wc1_pre[m]
                else:
                    wt = wc1p.tile([P, KX, P], MMDT, tag="wc1")
                    nc.sync.dma_start(wt[:],
                                      wc1_ext[m].rearrange("o p m -> p o m"))
                for hf in range(2):
                    ps = psp.tile([P, TH], F32, tag="mmps")
                    for k in range(KX):
                        nc.tensor.matmul(ps[:], lhsT=wt[:, k],
                                         rhs=xT_t[:, k, HSL[hf]],
                                         start=(k == 0), stop=(k == KX - 1))
                    chm = chq.tile([P, TH], MMDT, tag="chm")
                    nc.scalar.activation(chm[:], ps[:], AF.Gelu,
                                         bias=cpack_f[:, 1 + m:2 + m])
                    nc.tensor.matmul(psb[hf][:], lhsT=wc2_t[:, m],
                                     rhs=chm[:],
                                     start=(m == 0), stop=(m == MC1 - 1))

            # ctxpos rows: [0:64] ctx + b_c2, [64:96] pos_emb, [96:128] 0
            ctxpos, ctxpos_f = [], []
            for hf in range(2):
                cp = ctxq.tile([P, TH], MMDT, tag="ctxpos")
                nc.vector.tensor_scalar(cp[0:CTX, :], psb[hf][:],
                                        cpack_f[:CTX, 17:18], None,
                                        op0=ALU.add)
                nc.gpsimd.dma_start(cp[CTX:, :], posT_ext[:, HSL[hf]])
                cpf = cp if use_fp16 else cp.bitcast(F32)
                nc.vector.tensor_reduce(
                    ctxsum_sb[:, hf:hf + 1], cpf[0:CTX + POS_D, :],
                    axis=mybir.AxisListType.X, op=ALU.add)
                ctxpos.append(cp)
                ctxpos_f.append(cpf)

            def raw(hf, k):
                return (xT_t[:, k, HSL[hf]] if k < KX
                        else ctxpos[hf][:])

            def raw_f32(hf, k):
                return (xT_f32[:, k, HSL[hf]] if k < KX
                        else ctxpos_f[hf][:])

            # ---------- phase B: rms scale r (per half) ----------
            bcast_r = []
            for hf in range(2):
                ps_ss = pssp.tile([1, TH], F32, tag="smps")
                for k in range(KG):
                    sq = tmpq.tile([P, TH], MMDT, tag="tmp")
                    nc.vector.tensor_tensor(sq[:], raw_f32(hf, k),
                                            raw_f32(hf, k), ALU.mult)
                    nc.tensor.matmul(ps_ss[:], lhsT=ones_col, rhs=sq[:],
                                     start=(k == 0), stop=(k == KG - 1))
                rows = rowsp.tile([1, 3, TH], F32, tag="rows")
                s_row, t_row, r0 = rows[:, 0], rows[:, 1], rows[:, 2]
                nc.scalar.activation(s_row, ps_ss[:], AF.Copy,
                                     scale=1.0 / DG, bias=1e-6)
                nc.scalar.activation(t_row, s_row, AF.Sqrt)
                nc.vector.reciprocal(r0, t_row)
                # Newton: r = r0 * (1.5 - 0.5 * s * r0^2)
                nc.vector.tensor_tensor(t_row, r0, r0, ALU.mult)
                nc.vector.tensor_tensor(t_row, t_row, s_row, ALU.mult)
                nc.vector.tensor_scalar(t_row, t_row, -0.5, 1.5,
                                        op0=ALU.mult, op1=ALU.add)
                nc.vector.tensor_tensor(s_row, r0, t_row, ALU.mult)
                bc = bcq.tile([P, TH], F32, tag="bcast")
                nc.gpsimd.partition_broadcast(bc[:], s_row)
                bcast_r.append(bc)

            # ---------- phase C: u,v -> g, both halves per weight tile ----
            g_t = [gp.tile([P, MH, TH], F16, tag=f"g{hf}", name=f"g{hf}")
                   for hf in range(2)]
            for kh in range(MH):
                w13 = w13p.tile([P, 2, KG, P], MMDT, tag="w13")
                nc.sync.dma_start(w13[:, 0],
                                  w1_ext[kh].rearrange("o p m -> p o m"))
                nc.sync.dma_start(w13[:, 1],
                                  w3_ext[kh].rearrange("o p m -> p o m"))
                for hf in range(2):
                    psu = psp.tile([P, TH], F32, tag="mmps")
                    for k in range(KG):
                        nc.tensor.matmul(psu[:], lhsT=w13[:, 0, k],
                                         rhs=raw(hf, k),
                                         start=(k == 0), stop=(k == KG - 1))
                    psv = psp.tile([P, TH], F32, tag="mmps")
                    for k in range(KG):
                        nc.tensor.matmul(psv[:], lhsT=w13[:, 1, k],
                                         rhs=raw(hf, k),
                                         start=(k == 0), stop=(k == KG - 1))
                    us = tmpq.tile([P, TH], F32, tag="tmp")
                    nc.vector.tensor_tensor(us[:], psu[:], bcast_r[hf][:],
                                            ALU.mult)
                    su = suq.tile([P, TH], F32, tag="su")
                    nc.scalar.activation(su[:], us[:], AF.Silu)
                    nc.vector.tensor_tensor(su[:], su[:], psv[:], ALU.mult)
                    nc.vector.tensor_tensor(g_t[hf][:, kh], su[:],
                                            bcast_r[hf][:], ALU.mult)

            # ---------- phase D: h chunks + fused proj ----------
            psl = [pssp.tile([E, TH], F32, tag="smps", name=f"psl{i}")
                   for i in range(2)]
            for m in range(MO):
                w2ts = []
                for t3 in range(MH // W2T):
                    w2t = w2p.tile([P, W2T, P], F16, tag="w2")
                    nc.sync.dma_start(
                        w2t[:],
                        w2_ext[m, t3 * W2T:(t3 + 1) * W2T].rearrange(
                            "o p m -> p o m"))
                    w2ts.append(w2t)
                for hf in range(2):
                    psh = psp.tile([P, TH], F32, tag="mmps")
                    for k in range(MH):
                        nc.tensor.matmul(psh[:],
                                         lhsT=w2ts[k // W2T][:, k % W2T],
                                         rhs=g_t[hf][:, k],
                                         start=(k == 0), stop=(k == MH - 1))
                    us = tmpq.tile([P, TH], F32, tag="tmp")
                    nc.vector.scalar_tensor_tensor(
                        us[:], raw_f32(hf, m), cpack_f[:, 18 + m:19 + m],
                        bcast_r[hf][:], op0=ALU.mult, op1=ALU.mult)
                    hc = hcq.tile([P, TH], MMDT, tag="hc")
                    nc.vector.tensor_tensor(hc[:], psh[:], us[:], ALU.add)
                    nc.tensor.matmul(psl[hf][:], lhsT=proj_t[:, m],
                                     rhs=hc[:],
                                     start=(m == 0), stop=(m == MO - 1))
            for hf in range(2):
                nc.vector.tensor_copy(out=logits_sb[:, HSL[hf]],
                                      in_=psl[hf][:])

            nc.sync.dma_start(logits_ext[:], logits_sb[:])
            nc.sync.dma_start(ctxsum_ext[:], ctxsum_sb[:])

    nc.compile()
    return nc


def _get_module():
    key = "nc16" if USE_FP16 else "nc"
    if key not in _CACHE:
        _CACHE[key] = _build_module(USE_FP16)
    return _CACHE[key]


def _prep_weights(w_c1, b_c1, w_c2, b_c2, norm_w, mlp_w1, mlp_w2, mlp_w3,
                  proj_w, temperature):
    f = np.float32
    mdt = np.float16 if USE_FP16 else np.float32
    KX, MC1, KG, MH, MO = H // P, 2 * H // P, DGP // P, HID // P, DGP // P
    wc1b = np.ascontiguousarray(
        w_c1.astype(mdt).reshape(KX, P, MC1, P).transpose(2, 0, 1, 3))
    wc2b = np.ascontiguousarray(w_c2.astype(mdt).reshape(MC1, P, CTX))
    # fold norm_w into w1/w3 rows: w.T @ (raw*norm) == (norm*w).T @ raw
    nw = norm_w.astype(f)[:, None]
    w1p = np.zeros((DGP, HID), f)
    w1p[:DG] = mlp_w1.astype(f) * nw
    w1b = np.ascontiguousarray(
        w1p.astype(mdt).reshape(KG, P, MH, P).transpose(2, 0, 1, 3))
    w3p = np.zeros((DGP, HID), f)
    w3p[:DG] = mlp_w3.astype(f) * nw
    w3b = np.ascontiguousarray(
        w3p.astype(mdt).reshape(KG, P, MH, P).transpose(2, 0, 1, 3))
    w2p = np.zeros((HID, DGP), np.float16)
    w2p[:, :DG] = mlp_w2.astype(np.float16)
    w2b = np.ascontiguousarray(
        w2p.reshape(MH, P, MO, P).transpose(2, 0, 1, 3))
    temp = max(float(np.asarray(temperature)), 0.3)
    projp = np.zeros((DGP, E), f)
    projp[:DG] = proj_w.astype(f) / temp
    projb = np.ascontiguousarray(projp.astype(mdt).reshape(KG, P, E))
    bc1 = np.ascontiguousarray(b_c1.astype(f).reshape(MC1, P).T)
    bc2 = np.ascontiguousarray(b_c2.astype(f).reshape(CTX, 1))
    normp = np.zeros((DGP,), f)
    normp[:DG] = norm_w.astype(f)
    normc = np.ascontiguousarray(normp.reshape(KG, P).T)
    return dict(wc1b=wc1b, wc2b=wc2b, w1b=w1b, w3b=w3b, w2b=w2b,
                projb=projb, bc1=bc1, bc2=bc2, normc=normc,
                ones=np.ones((P, 1), mdt))


def kernel(x, positions, w_c1, b_c1, w_c2, b_c2, norm_w, mlp_w1, mlp_w2,
           mlp_w3, proj_w, pos_table, temperature):
    from concourse.bass_utils import run_bass_kernel_spmd

    nc = _get_module()
    f = np.float32
    x_flat = np.asarray(x, f).reshape(N, H)
    pos_flat = np.asarray(positions).reshape(N)
    pos_table = np.asarray(pos_table, f)
    wmaps = _prep_weights(w_c1, b_c1, w_c2, b_c2, norm_w, mlp_w1, mlp_w2,
                          mlp_w3, proj_w, temperature)

    in_maps = []
    for c in range(NCORES):
        sl = slice(c * T, (c + 1) * T)
        mdt = np.float16 if USE_FP16 else np.float32
        m = dict(wmaps)
        m["xT"] = np.ascontiguousarray(x_flat[sl].T.astype(mdt))
        posT = np.zeros((P - CTX, T), mdt)
        posT[:POS_D] = pos_table[pos_flat[sl]].T
        m["posT"] = posT
        in_maps.append(m)

    res = run_bass_kernel_spmd(nc, in_maps, list(range(NCORES)))

    logits = np.concatenate([res.results[c]["logits"].T
                             for c in range(NCORES)], axis=0)   # [N, E]
    ctxsums = np.stack([res.results[c]["ctxsum"].sum(axis=1)
                        for c in range(NCORES)], axis=0)        # [8, 96]

    # ---- host routing tail (exact reference semantics) ----
    z = logits - logits.max(axis=-1, keepdims=True)
    ez = np.exp(z)
    scores = (ez / ez.sum(axis=-1, keepdims=True)).astype(f)

    order2 = np.argsort(-scores, axis=-1, kind="stable")
    topk_idx = order2[:, :K].astype(np.int32)
    topk_scores = np.take_along_axis(scores, topk_idx, axis=-1).astype(f)

    fe = topk_idx.reshape(-1)
    fw = topk_scores.reshape(-1)
    order = np.argsort(-fw, kind="stable")
    e_sorted = fe[order]
    oh = np.zeros((N * K, E), np.int32)
    oh[np.arange(N * K), e_sorted] = 1
    rank = np.take_along_axis(np.cumsum(oh, axis=0),
                              e_sorted[:, None], axis=1)[:, 0] - 1
    ok = rank < CAPACITY
    slot_sorted = np.where(ok, rank, -1).astype(np.int32)
    slot = np.zeros(N * K, np.int32)
    slot[order] = slot_sorted
    assigned = np.zeros(N * K, bool)
    assigned[order] = ok
    assigned_mask = assigned.reshape(N, K)
    buffer_positions = np.maximum(slot, 0).reshape(N, K).astype(np.int32)
    overflow_mask = ~assigned_mask.any(axis=-1)
    expert_count = np.bincount(fe, weights=assigned.astype(np.float64),
                               minlength=E).astype(np.int32)

    me = scores.mean(axis=0)
    ce = expert_count.astype(f) / f(N)
    aux_loss = f(AUX_COEF * E * np.sum(me * ce, dtype=f))

    next_context = np.zeros((B, CTX + POS_D), f)
    for b in range(B):
        next_context[b] = (ctxsums[2 * b] + ctxsums[2 * b + 1]) / f(S)

    return (topk_idx, topk_scores, assigned_mask, buffer_positions,
            overflow_mask, expert_count, np.float32(aux_loss), next_context)


# revision 23
# speedup vs baseline: 1.0744x; 1.0744x over previous
"""ByteContextAwareRouter Trainium2 kernel.

Strategy: data-parallel over the N=B*S=8192 token axis across 8 NeuronCores
(1024 tokens/core). Each core runs the full gate pipeline feature-major
(activations stored [feature, token]) so no activation transposes are needed:

  ctx_hidden = gelu(x @ w_c1 + b_c1)        [2048, T]
  ctx        = ctx_hidden @ w_c2 + b_c2     [64, T]
  gi         = rmsnorm([x; ctx; pos_emb])   [1152(pad), T]
  h          = (silu(gi@w1) * (gi@w3)) @ w2 + gi
  logits     = h @ (proj_w / max(temp,.3))  [8, T]

Matmuls run in float32r (full-rate reduced-precision fp32, ~1.4e-4 max rel
err measured on HW) with fp32 PSUM accumulation. The per-token rmsnorm scale
r commutes through the w1/w3 matmuls, so those take the raw resident
[x; ctx; pos] chunks as RHS and r is applied at PSUM eviction — no
materialized gi buffer. Tokens are processed in two 512-token passes so all
intermediates fit in SBUF; weights stream from HBM once per pass. Device
outputs per core: logits [8,1024] and context-feature column sums [96,2].
The tiny routing tail (softmax, top-2, global sorted greedy capacity
dispatch over 16384 pairs, aux loss) replicates the reference exactly on
host in numpy.
"""
import numpy as np

B, S, H = 4, 2048, 1024
CTX, E, K = 64, 8, 2
POS_D = 32
DG = 1120          # H + CTX + POS_D
DGP = 1152         # padded to 9*128
HID = 3072
N = B * S
CAPACITY = 2048
AUX_COEF = 0.01
NCORES = 8
T = N // NCORES    # 1024 tokens per core
TH = 512           # tokens per pass (matmul free dim)
P = 128

_CACHE = {}
USE_FP16 = False


def _build_module(use_fp16=False):
    import concourse.bacc as bacc
    import concourse.mybir as mybir
    from concourse.tile import TileContext

    F32 = mybir.dt.float32
    F16 = mybir.dt.float16
    MMDT = F16 if use_fp16 else mybir.dt.float32r
    AF = mybir.ActivationFunctionType
    ALU = mybir.AluOpType

    KX = H // P          # 8   x feature chunks
    MC1 = 2 * H // P     # 16  ctx-hidden chunks
    KG = DGP // P        # 9   gate-input chunks
    MH = HID // P        # 24  hidden chunks
    MO = DGP // P        # 9   mlp output chunks
    W2T = 8              # w2 k-chunks per streamed tile

    nc = bacc.Bacc("TRN2", target_bir_lowering=False, debug=False,
                   num_devices=NCORES)

    # ---- DRAM parameters (per-core shards / replicated weight blobs) ----
    xT_ext = nc.declare_dram_parameter("xT", [H, T], MMDT, isOutput=False)
    # pos_emb.T padded with zero rows to fill ctxpos partitions 64..127
    posT_ext = nc.declare_dram_parameter("posT", [P - CTX, T], MMDT,
                                         isOutput=False)
    wc1_ext = nc.declare_dram_parameter("wc1b", [MC1, KX, P, P], MMDT,
                                        isOutput=False)
    wc2_ext = nc.declare_dram_parameter("wc2b", [MC1, P, CTX], MMDT,
                                        isOutput=False)
    w1_ext = nc.declare_dram_parameter("w1b", [MH, KG, P, P], MMDT,
                                       isOutput=False)
    w3_ext = nc.declare_dram_parameter("w3b", [MH, KG, P, P], MMDT,
                                       isOutput=False)
    w2_ext = nc.declare_dram_parameter("w2b", [MO, MH, P, P], F16,
                                       isOutput=False)
    proj_ext = nc.declare_dram_parameter("projb", [KG, P, E], MMDT,
                                         isOutput=False)
    ones_ext = nc.declare_dram_parameter("ones", [P, 1], MMDT, isOutput=False)
    bc1_ext = nc.declare_dram_parameter("bc1", [P, MC1], F32, isOutput=False)
    bc2_ext = nc.declare_dram_parameter("bc2", [CTX, 1], F32, isOutput=False)
    norm_ext = nc.declare_dram_parameter("normc", [P, KG], F32,
                                         isOutput=False)
    logits_ext = nc.declare_dram_parameter("logits", [E, T], F32,
                                           isOutput=True)
    ctxsum_ext = nc.declare_dram_parameter("ctxsum", [CTX + POS_D, 2], F32,
                                           isOutput=True)

    from contextlib import ExitStack
    with TileContext(nc) as tc:
        with ExitStack() as stack:
            ep = stack.enter_context
            constp = ep(tc.tile_pool(name="const", bufs=1))
            xp = ep(tc.tile_pool(name="xT", bufs=1))
            gp = ep(tc.tile_pool(name="g", bufs=1))
            hcq = ep(tc.tile_pool(name="hc", bufs=2))
            wc1p = ep(tc.tile_pool(name="wc1s", bufs=3))
            w13p = ep(tc.tile_pool(name="w13s", bufs=3))
            w2p = ep(tc.tile_pool(name="w2s", bufs=4))
            ctxq = ep(tc.tile_pool(name="ctxq", bufs=2))
            chq = ep(tc.tile_pool(name="chq", bufs=2))
            tmpq = ep(tc.tile_pool(name="tmpq", bufs=2))
            suq = ep(tc.tile_pool(name="suq", bufs=2))
            bcq = ep(tc.tile_pool(name="bcq", bufs=2))
            rowsp = ep(tc.tile_pool(name="rows", bufs=1))
            outp = ep(tc.tile_pool(name="out", bufs=1))
            psp = ep(tc.tile_pool(name="psum", bufs=6, space="PSUM"))
            pssp = ep(tc.tile_pool(name="psum_s", bufs=2, space="PSUM"))
            # ---- packed f32 constants: 1:17=bc1, 17=bc2, 18:27=norm ----
            cpack_f = constp.tile([P, 28], F32)
            nc.sync.dma_start(cpack_f[:, 1:1 + MC1], bc1_ext[:])
            nc.sync.dma_start(cpack_f[:CTX, 17:18], bc2_ext[:])
            nc.sync.dma_start(cpack_f[:, 18:18 + KG], norm_ext[:])

            # first two w_c1 tiles ahead of the bulk xT load so the PE can
            # start as soon as the first xT chunk lands
            wc1_pre = []
            for m in range(2):
                wt = wc1p.tile([P, KX, P], MMDT, tag="wc1",
                               name=f"wc1pre{m}")
                nc.sync.dma_start(wt[:],
                                  wc1_ext[m].rearrange("o p m -> p o m"))
                wc1_pre.append(wt)

            xT_ch = xT_ext.rearrange("(o p) t -> o p t", p=P)
            xT_t = xp.tile([P, KX, T], MMDT)
            for k in range(KX):
                nc.sync.dma_start(xT_t[:, k], xT_ch[k])
            xT_f32 = xT_t if use_fp16 else xT_t.bitcast(F32)

            wc2_t = constp.tile([P, MC1, CTX], MMDT)
            nc.sync.dma_start(wc2_t[:], wc2_ext.rearrange("o p n -> p o n"))
            proj_t = constp.tile([P, KG, E], MMDT)
            nc.sync.dma_start(proj_t[:], proj_ext.rearrange("o p n -> p o n"))
            ones_t = constp.tile([P, 1], MMDT)
            nc.sync.dma_start(ones_t[:], ones_ext[:])
            ones_col = ones_t[:]

            logits_sb = outp.tile([E, T], F32)
            ctxsum_sb = outp.tile([CTX + POS_D, 2], F32)

            HSL = [slice(0, TH), slice(TH, 2 * TH)]

            # ---------- phase A: ctx MLP, both halves per weight tile -----
            psb = [pssp.tile([CTX, TH], F32, tag="smps", name=f"psb{i}")
                   for i in range(2)]
            for m in range(MC1):
                if m < 2:
                    wt = wc1_pre[m]
                else:
                    wt = wc1p.tile([P, KX, P], MMDT, tag="wc1")
                    nc.sync.dma_start(wt[:],
                                      wc1_ext[m].rearrange("o p m -> p o m"))
                for hf in range(2):
                    ps = psp.tile([P, TH], F32, tag="mmps")
                    for k in range(KX):
                        nc.tensor.matmul(ps[:], lhsT=wt[:, k],
                                         rhs=xT_t[:, k, HSL[hf]],
                                         start=(k == 0), stop=(k == KX - 1))
                    chm = chq.tile([P, TH], MMDT, tag="chm")
                    nc.scalar.activation(chm[:], ps[:], AF.Gelu,
                                         bias=cpack_f[:, 1 + m:2 + m])
                    nc.tensor.matmul(psb[hf][:], lhsT=wc2_t[:, m],
                                     rhs=chm[:],
                                     start=(m == 0), stop=(m == MC1 - 1))

            # ctxpos rows: [0:64] ctx + b_c2, [64:96] pos_emb, [96:128] 0
            ctxpos, ctxpos_f = [], []
            for hf in range(2):
                cp = ctxq.tile([P, TH], MMDT, tag="ctxpos")
                nc.vector.tensor_scalar(cp[0:CTX, :], psb[hf][:],
                                        cpack_f[:CTX, 17:18], None,
                                        op0=ALU.add)
                nc.sync.dma_start(cp[CTX:, :], posT_ext[:, HSL[hf]])
                cpf = cp if use_fp16 else cp.bitcast(F32)
                nc.vector.tensor_reduce(
                    ctxsum_sb[:, hf:hf + 1], cpf[0:CTX + POS_D, :],
                    axis=mybir.AxisListType.X, op=ALU.add)
                ctxpos.append(cp)
                ctxpos_f.append(cpf)

            def raw(hf, k):
                return (xT_t[:, k, HSL[hf]] if k < KX
                        else ctxpos[hf][:])

            def raw_f32(hf, k):
                return (xT_f32[:, k, HSL[hf]] if k < KX
                        else ctxpos_f[hf][:])

            # ---------- phase B: rms scale r (per half) ----------
            bcast_r = []
            for hf in range(2):
                ps_ss = pssp.tile([1, TH], F32, tag="smps")
                for k in range(KG):
                    sq = tmpq.tile([P, TH], MMDT, tag="tmp")
                    nc.vector.tensor_tensor(sq[:], raw_f32(hf, k),
                                            raw_f32(hf, k), ALU.mult)
                    nc.tensor.matmul(ps_ss[:], lhsT=ones_col, rhs=sq[:],
                                     start=(k == 0), stop=(k == KG - 1))
                rows = rowsp.tile([1, 3, TH], F32, tag="rows")
                s_row, t_row, r0 = rows[:, 0], rows[:, 1], rows[:, 2]
                nc.scalar.activation(s_row, ps_ss[:], AF.Copy,
                                     scale=1.0 / DG, bias=1e-6)
                nc.scalar.activation(t_row, s_row, AF.Sqrt)
                nc.vector.reciprocal(r0, t_row)
                # Newton: r = r0 * (1.5 - 0.5 * s * r0^2)
                nc.vector.tensor_tensor(t_row, r0, r0, ALU.mult)
                nc.vector.tensor_tensor(t_row, t_row, s_row, ALU.mult)
                nc.vector.tensor_scalar(t_row, t_row, -0.5, 1.5,
                                        op0=ALU.mult, op1=ALU.add)
                nc.vector.tensor_tensor(s_row, r0, t_row, ALU.mult)
                bc = bcq.tile([P, TH], F32, tag="bcast")
                nc.gpsimd.partition_broadcast(bc[:], s_row)
                bcast_r.append(bc)

            # ---------- phase C: u,v -> g, both halves per weight tile ----
            g_t = [gp.tile([P, MH, TH], F16, tag=f"g{hf}", name=f"g{hf}")
                   for hf in range(2)]
            for kh in range(MH):
                w13 = w13p.tile([P, 2, KG, P], MMDT, tag="w13")
                nc.sync.dma_start(w13[:, 0],
                                  w1_ext[kh].rearrange("o p m -> p o m"))
                nc.sync.dma_start(w13[:, 1],
                                  w3_ext[kh].rearrange("o p m -> p o m"))
                for hf in range(2):
                    psu = psp.tile([P, TH], F32, tag="mmps")
                    for k in range(KG):
                        nc.tensor.matmul(psu[:], lhsT=w13[:, 0, k],
                                         rhs=raw(hf, k),
                                         start=(k == 0), stop=(k == KG - 1))
                    psv = psp.tile([P, TH], F32, tag="mmps")
                    for k in range(KG):
                        nc.tensor.matmul(psv[:], lhsT=w13[:, 1, k],
                                         rhs=raw(hf, k),
                                         start=(k == 0), stop=(k == KG - 1))
                    us = tmpq.tile([P, TH], F32, tag="tmp")
                    nc.vector.tensor_tensor(us[:], psu[:], bcast_r[hf][:],
                                            ALU.mult)
                    su = suq.tile([P, TH], F32, tag="su")
                    nc.scalar.activation(su[:], us[:], AF.Silu)
                    nc.vector.tensor_tensor(su[:], su[:], psv[:], ALU.mult)
                    nc.vector.tensor_tensor(g_t[hf][:, kh], su[:],
                                            bcast_r[hf][:], ALU.mult)

            # ---------- phase D: h chunks + fused proj ----------
            psl = [pssp.tile([E, TH], F32, tag="smps", name=f"psl{i}")
                   for i in range(2)]
            for m in range(MO):
                w2ts = []
                for t3 in range(MH // W2T):
                    w2t = w2p.tile([P, W2T, P], F16, tag="w2")
                    nc.sync.dma_start(
                        w2t[:],
                        w2_ext[m, t3 * W2T:(t3 + 1) * W2T].rearrange(
                            "o p m -> p o m"))
                    w2ts.append(w2t)
                for hf in range(2):
                    psh = psp.tile([P, TH], F32, tag="mmps")
                    for k in range(MH):
                        nc.tensor.matmul(psh[:],
                                         lhsT=w2ts[k // W2T][:, k % W2T],
                                         rhs=g_t[hf][:, k],
                                         start=(k == 0), stop=(k == MH - 1))
                    us = tmpq.tile([P, TH], F32, tag="tmp")
                    nc.vector.scalar_tensor_tensor(
                        us[:], raw_f32(hf, m), cpack_f[:, 18 + m:19 + m],
                        bcast_r[hf][:], op0=ALU.mult, op1=ALU.mult)
                    hc = hcq.tile([P, TH], MMDT, tag="hc")
                    nc.vector.tensor_tensor(hc[:], psh[:], us[:], ALU.add)
                    nc.tensor.matmul(psl[hf][:], lhsT=proj_t[:, m],
                                     rhs=hc[:],
                                     start=(m == 0), stop=(m == MO - 1))
            for hf in range(2):
                nc.vector.tensor_copy(out=logits_sb[:, HSL[hf]],
                                      in_=psl[hf][:])

            nc.sync.dma_start(logits_ext[:], logits_sb[:])
            nc.sync.dma_start(ctxsum_ext[:], ctxsum_sb[:])

    nc.compile()
    return nc


def _get_module():
    key = "nc16" if USE_FP16 else "nc"
    if key not in _CACHE:
        _CACHE[key] = _build_module(USE_FP16)
    return _CACHE[key]


def _prep_weights(w_c1, b_c1, w_c2, b_c2, norm_w, mlp_w1, mlp_w2, mlp_w3,
                  proj_w, temperature):
    f = np.float32
    mdt = np.float16 if USE_FP16 else np.float32
    KX, MC1, KG, MH, MO = H // P, 2 * H // P, DGP // P, HID // P, DGP // P
    wc1b = np.ascontiguousarray(
        w_c1.astype(mdt).reshape(KX, P, MC1, P).transpose(2, 0, 1, 3))
    wc2b = np.ascontiguousarray(w_c2.astype(mdt).reshape(MC1, P, CTX))
    # fold norm_w into w1/w3 rows: w.T @ (raw*norm) == (norm*w).T @ raw
    nw = norm_w.astype(f)[:, None]
    w1p = np.zeros((DGP, HID), f)
    w1p[:DG] = mlp_w1.astype(f) * nw
    w1b = np.ascontiguousarray(
        w1p.astype(mdt).reshape(KG, P, MH, P).transpose(2, 0, 1, 3))
    w3p = np.zeros((DGP, HID), f)
    w3p[:DG] = mlp_w3.astype(f) * nw
    w3b = np.ascontiguousarray(
        w3p.astype(mdt).reshape(KG, P, MH, P).transpose(2, 0, 1, 3))
    w2p = np.zeros((HID, DGP), np.float16)
    w2p[:, :DG] = mlp_w2.astype(np.float16)
    w2b = np.ascontiguousarray(
        w2p.reshape(MH, P, MO, P).transpose(2, 0, 1, 3))
    temp = max(float(np.asarray(temperature)), 0.3)
    projp = np.zeros((DGP, E), f)
    projp[:DG] = proj_w.astype(f) / temp
    projb = np.ascontiguousarray(projp.astype(mdt).reshape(KG, P, E))
    bc1 = np.ascontiguousarray(b_c1.astype(f).reshape(MC1, P).T)
    bc2 = np.ascontiguousarray(b_c2.astype(f).reshape(CTX, 1))
    normp = np.zeros((DGP,), f)
    normp[:DG] = norm_w.astype(f)
    normc = np.ascontiguousarray(normp.reshape(KG, P).T)
    return dict(wc1b=wc1b, wc2b=wc2b, w1b=w1b, w3b=w3b, w2b=w2b,
                projb=projb, bc1=bc1, bc2=bc2, normc=normc,
                ones=np.ones((P, 1), mdt))


def kernel(x, positions, w_c1, b_c1, w_c2, b_c2, norm_w, mlp_w1, mlp_w2,
           mlp_w3, proj_w, pos_table, temperature):
    from concourse.bass_utils import run_bass_kernel_spmd

    nc = _get_module()
    f = np.float32
    x_flat = np.asarray(x, f).reshape(N, H)
    pos_flat = np.asarray(positions).reshape(N)
    pos_table = np.asarray(pos_table, f)
    wmaps = _prep_weights(w_c1, b_c1, w_c2, b_c2, norm_w, mlp_w1, mlp_w2,
                          mlp_w3, proj_w, temperature)

    in_maps = []
    for c in range(NCORES):
        sl = slice(c * T, (c + 1) * T)
        mdt = np.float16 if USE_FP16 else np.float32
        m = dict(wmaps)
        m["xT"] = np.ascontiguousarray(x_flat[sl].T.astype(mdt))
        posT = np.zeros((P - CTX, T), mdt)
        posT[:POS_D] = pos_table[pos_flat[sl]].T
        m["posT"] = posT
        in_maps.append(m)

    res = run_bass_kernel_spmd(nc, in_maps, list(range(NCORES)))

    logits = np.concatenate([res.results[c]["logits"].T
                             for c in range(NCORES)], axis=0)   # [N, E]
    ctxsums = np.stack([res.results[c]["ctxsum"].sum(axis=1)
                        for c in range(NCORES)], axis=0)        # [8, 96]

    # ---- host routing tail (exact reference semantics) ----
    z = logits - logits.max(axis=-1, keepdims=True)
    ez = np.exp(z)
    scores = (ez / ez.sum(axis=-1, keepdims=True)).astype(f)

    order2 = np.argsort(-scores, axis=-1, kind="stable")
    topk_idx = order2[:, :K].astype(np.int32)
    topk_scores = np.take_along_axis(scores, topk_idx, axis=-1).astype(f)

    fe = topk_idx.reshape(-1)
    fw = topk_scores.reshape(-1)
    order = np.argsort(-fw, kind="stable")
    e_sorted = fe[order]
    oh = np.zeros((N * K, E), np.int32)
    oh[np.arange(N * K), e_sorted] = 1
    rank = np.take_along_axis(np.cumsum(oh, axis=0),
                              e_sorted[:, None], axis=1)[:, 0] - 1
    ok = rank < CAPACITY
    slot_sorted = np.where(ok, rank, -1).astype(np.int32)
    slot = np.zeros(N * K, np.int32)
    slot[order] = slot_sorted
    assigned = np.zeros(N * K, bool)
    assigned[order] = ok
    assigned_mask = assigned.reshape(N, K)
    buffer_positions = np.maximum(slot, 0).reshape(N, K).astype(np.int32)
    overflow_mask = ~assigned_mask.any(axis=-1)
    expert_count = np.bincount(fe, weights=assigned.astype(np.float64),
                               minlength=E).astype(np.int32)

    me = scores.mean(axis=0)
    ce = expert_count.astype(f) / f(N)
    aux_loss = f(AUX_COEF * E * np.sum(me * ce, dtype=f))

    next_context = np.zeros((B, CTX + POS_D), f)
    for b in range(B):
        next_context[b] = (ctxsums[2 * b] + ctxsums[2 * b + 1]) / f(S)

    return (topk_idx, topk_scores, assigned_mask, buffer_positions,
            overflow_mask, expert_count, np.float32(aux_loss), next_context)
